# revision 11
# baseline (speedup 1.0000x reference)
"""Karras optimal denoiser on 8 Trainium2 NeuronCores.

Math: D(x, sigma) = softmax_i(L_bi) @ y  with  L_bi = (x_b . y_i - 0.5||y_i||^2) / sigma_b^2
(the per-row constant -0.5||x||^2/sigma^2 cancels in softmax).

Sharding: train_data split over N across 8 cores. Each core returns
(num, den, mx) = (sum_i w y_i, sum_i w, rowmax L * invsig2) in its local max
frame; host does the flash-style combine.

Per-core kernel:
  GEMM1 (logits): single-pass fp16  x.y  accumulated in fp32 PSUM (CPU sim of
    this exact pipeline gives rel err ~3e-3 vs the 2e-2 gate); -0.5||y||^2 is
    added via a K=2 fp16 ones-matmul broadcast of the (hi,lo) fp16 split of y2.
  Softmax: per-chunk DVE rowmax + ACT exp (scale=1/sigma^2, bias=-chunkmax/sigma^2,
    accum_out=rowsum); after all chunks, per-chunk correction factors
    exp((cm_c - gm)/sigma^2) rescale W (m=0 on DVE, m=1 on ACT in parallel),
    each chunk's transposes issued right behind its rescale.
  W^T via PE transpose-mode (fp16), then GEMM2 (num = W @ y) with W^T stationary
    and native-layout fp16 y as the moving operand, triple-buffered; the first
    two y-chunks stream in small pieces during GEMM1 (phase-1 DMA is ~80%
    saturated by the y^T stream, lumpy prefetch stalls the PE), the third at
    the tail, the rest 3-deep during GEMM2.
"""

import numpy as np
from contextlib import ExitStack

import concourse.bass as bass
import concourse.tile as tile
import concourse.mybir as mybir
from concourse import bacc
from concourse.bass_utils import run_bass_kernel_spmd
from concourse.masks import make_identity

dt = mybir.dt

B, C, H, W_IMG = 256, 3, 32, 32
D = C * H * W_IMG            # 3072
N_TOTAL = 50000
N_CORES = 8
NS = N_TOTAL // N_CORES      # 6250 per core
KT = D // 128                # 24 contraction k-tiles
HK = KT // 2                 # 12 k-tiles per xs half (startup pipelining)
M_TILES = 2                  # 256 query rows = 2 partition tiles
CH1 = 384                    # GEMM1 chunk width (SBUF-driven)
G2CH = 256                   # GEMM2 output-d chunk width
DCH2 = D // G2CH             # 12 output-d chunks for GEMM2
YNBUF = 3                    # GEMM2 y stream depth


def ceil128(v):
    return (v + 127) // 128 * 128


def chunk_list(ns):
    """Leading small chunks shrink the first-matmul DMA wait."""
    out = []
    off = 0
    for sz in (128, 128, 256):
        out.append((off, sz))
        off += sz
    while off < ns:
        sz = min(CH1, ns - off)
        out.append((off, sz))
        off += sz
    return out


def build_nc(ns=NS):
    """Build + compile the per-core Bass program (parameterized by shard size)."""
    chunks = chunk_list(ns)
    nch = len(chunks)
    ki_n = (ns + 127) // 128          # ki tiles for GEMM2 (ynat zero-padded)
    ns_pad = ki_n * 128
    assert sum(ceil128(csz) for _, csz in chunks) == ns_pad
    # ki tile -> (chunk idx, col offset inside chunk tile)
    ki_map = []
    for ci, (coff, csz) in enumerate(chunks):
        for lo in range(0, ceil128(csz), 128):
            ki_map.append((ci, lo))
    assert len(ki_map) == ki_n

    nc = bacc.Bacc("TRN2", target_bir_lowering=False, debug=False)

    # --- DRAM I/O ---
    yh_d = [nc.dram_tensor(f"yh_c{ci}", (KT, 128, csz), dt.float16, kind="ExternalInput").ap()
            for ci, (_, csz) in enumerate(chunks)]
    y2_d = nc.dram_tensor("y2hl", (2, ns), dt.float16, kind="ExternalInput").ap()
    xs_d = nc.dram_tensor("xs", (M_TILES, KT, 128, 128), dt.float16, kind="ExternalInput").ap()
    yn_d = nc.dram_tensor("ynat", (ns_pad, D), dt.float16, kind="ExternalInput").ap()
    is2_d = nc.dram_tensor("invsig2", (M_TILES, 128), dt.float32, kind="ExternalInput").ap()

    num_d = nc.dram_tensor("num", (M_TILES, 128, D), dt.float32, kind="ExternalOutput").ap()
    den_d = nc.dram_tensor("den", (M_TILES, 128, 1), dt.float32, kind="ExternalOutput").ap()
    mx_d = nc.dram_tensor("mx", (M_TILES, 128, 1), dt.float32, kind="ExternalOutput").ap()

    with tile.TileContext(nc) as tc:
        with ExitStack() as ctx:
            small = ctx.enter_context(tc.tile_pool(name="small", bufs=1))
            xpool = ctx.enter_context(tc.tile_pool(name="x", bufs=1))
            wpool = ctx.enter_context(tc.tile_pool(name="w", bufs=1))
            wtpool = ctx.enter_context(tc.tile_pool(name="wt", bufs=1))
            ynpool = ctx.enter_context(tc.tile_pool(name="yn", bufs=1))
            biasp = ctx.enter_context(tc.tile_pool(name="bias", bufs=8))
            g1ps = ctx.enter_context(tc.tile_pool(name="g1ps", bufs=5, space="PSUM"))
            tps = ctx.enter_context(tc.tile_pool(name="tps", bufs=3, space="PSUM"))
            outp = ctx.enter_context(tc.tile_pool(name="odrain", bufs=6))
            ypool = ctx.enter_context(tc.tile_pool(name="ystream", bufs=2))
            y2pool = ctx.enter_context(tc.tile_pool(name="y2p", bufs=2))

            # ---- chunk-0 stream DMAs first so the PE can start ASAP ----
            def load_chunk(ci):
                coff, csz = chunks[ci]
                yh_t = ypool.tile([128, KT, csz], dt.float16, tag="yh")
                y2_t = y2pool.tile([2, csz], dt.float16, tag="y2")
                nc.sync.dma_start(yh_t[:], yh_d[ci].rearrange("k p s -> p k s"))
                nc.sync.dma_start(y2_t[:], y2_d[:, coff:coff + csz])
                return yh_t, y2_t

            pending = load_chunk(0)

            # constants / small state
            is2_t = small.tile([128, M_TILES], dt.float32, tag="is2")
            nc.sync.dma_start(is2_t[:], is2_d.rearrange("m p -> p m"))
            xs_t = [[xpool.tile([128, HK, 128], dt.float16, tag=f"xs{m}h{h}",
                                name=f"xs{m}h{h}") for h in range(2)]
                    for m in range(M_TILES)]
            for m in range(M_TILES):
                for h in range(2):
                    nc.sync.dma_start(xs_t[m][h][:],
                                      xs_d[m, h * HK:(h + 1) * HK].rearrange("k p b -> p k b"))

            ident = small.tile([128, 128], dt.float16, tag="ident")
            make_identity(nc, ident[:])
            ones2 = small.tile([2, 128], dt.float16, tag="ones2")
            nc.vector.memset(ones2[:], 1.0)
            cm_st = [small.tile([128, nch], dt.float32, tag=f"cm{m}", name=f"cm{m}") for m in range(M_TILES)]
            s_st = [small.tile([128, nch], dt.float32, tag=f"ss{m}", name=f"ss{m}") for m in range(M_TILES)]
            fac = [small.tile([128, nch], dt.float32, tag=f"fac{m}", name=f"fac{m}") for m in range(M_TILES)]
            scr = [small.tile([128, nch], dt.float32, tag=f"scr{m}", name=f"scr{m}") for m in range(M_TILES)]
            # per-chunk W tiles (fine-grained rescale -> transpose pipelining)
            wc_t = [[wpool.tile([128, ceil128(csz)], dt.float16, tag=f"W{m}c{ci}",
                                name=f"W{m}c{ci}") for ci, (_, csz) in enumerate(chunks)]
                    for m in range(M_TILES)]
            wt_t = [wtpool.tile([128, M_TILES * 128], dt.float16, tag=f"wt{t}", name=f"wt{t}") for t in range(ki_n)]
            yn_t = [ynpool.tile([128, ki_n, G2CH], dt.float16, tag=f"yn{j}", name=f"yn{j}")
                    for j in range(YNBUF)]

            for m in range(M_TILES):
                for ci, (_, csz) in enumerate(chunks):
                    if ceil128(csz) > csz:
                        nc.vector.memset(wc_t[m][ci][:, csz:], 0.0)

            def yn_dma(j, dci, ki0=0, ki1=ki_n):
                nc.sync.dma_start(
                    yn_t[j][:, ki0:ki1, :],
                    yn_d[ki0 * 128:ki1 * 128,
                         dci * G2CH:(dci + 1) * G2CH].rearrange(
                        "(ki p) s -> p ki s", p=128))

            # spread the first GEMM2 y-chunk load thinly across phase 1
            # (phase-1 DMA runs at ~90% of roofline; lumpy prefetch stalls PE)
            pieces = []                 # (buf j, dci, ki0, ki1)
            for s in range(10):
                ki0 = (ki_n * s) // 10
                ki1 = (ki_n * (s + 1)) // 10
                if ki1 > ki0:
                    pieces.append((0, 0, ki0, ki1))
            first_pf = 6                # chunk idx of first prefetch piece

            # ---------------- GEMM1 + per-chunk softmax pieces ----------------
            for ci, (coff, csz) in enumerate(chunks):
                yh_t, y2_t = pending
                if ci + 1 < nch:
                    pending = load_chunk(ci + 1)
                if first_pf <= ci < first_pf + len(pieces):
                    yn_dma(*pieces[ci - first_pf])

                for m in range(M_TILES):
                    ps = g1ps.tile([128, CH1], dt.float32, tag="g1ps")
                    psv = ps[:, :csz]
                    # exact -0.5*||y||^2 broadcast: ones[2,128].T @ y2hl[2,csz] (fp16)
                    nc.tensor.matmul(psv, ones2[:, :], y2_t[:, :], start=True, stop=False)
                    for k in range(KT):
                        nc.tensor.matmul(
                            psv,
                            xs_t[m][k // HK][:, k % HK, :],
                            yh_t[:, k, :],
                            start=False,
                            stop=(k == KT - 1),
                        )
                    cmsl = cm_st[m][:, ci:ci + 1]
                    nc.vector.reduce_max(cmsl, psv, mybir.AxisListType.X)
                    bias_t = biasp.tile([128, 1], dt.float32, tag="bias")
                    nc.vector.tensor_scalar(
                        bias_t[:], cmsl, is2_t[:, m:m + 1], -1.0,
                        op0=mybir.AluOpType.mult, op1=mybir.AluOpType.mult)
                    nc.scalar.activation(
                        wc_t[m][ci][:, :csz], psv,
                        mybir.ActivationFunctionType.Exp,
                        bias=bias_t[:], scale=is2_t[:, m:m + 1],
                        accum_out=s_st[m][:, ci:ci + 1])

            # issue any prefetch pieces that didn't fit in the chunk loop
            for p in pieces[max(0, nch - first_pf):]:
                yn_dma(*p)
            # fill the tail DMA window with the next two GEMM2 y chunks
            yn_dma(1, 1)
            yn_dma(2, 2)

            # ------------- global max, factors, den/mx -------------
            for m in range(M_TILES):
                gm = small.tile([128, 1], dt.float32, tag=f"gm{m}")
                nc.vector.reduce_max(gm[:], cm_st[m][:, :], mybir.AxisListType.X)
                neg = small.tile([128, 1], dt.float32, tag=f"neg{m}")
                nc.vector.tensor_scalar(
                    neg[:], gm[:], is2_t[:, m:m + 1], -1.0,
                    op0=mybir.AluOpType.mult, op1=mybir.AluOpType.mult)
                mxs = small.tile([128, 1], dt.float32, tag=f"mxs{m}")
                nc.vector.tensor_scalar_mul(mxs[:], neg[:], -1.0)
                nc.sync.dma_start(mx_d[m], mxs[:])
                nc.scalar.activation(
                    fac[m][:, :], cm_st[m][:, :],
                    mybir.ActivationFunctionType.Exp,
                    bias=neg[:], scale=is2_t[:, m:m + 1])
                dsb = small.tile([128, 1], dt.float32, tag=f"den{m}")
                nc.vector.tensor_mul(scr[m][:, :], s_st[m][:, :], fac[m][:, :])
                nc.vector.reduce_sum(dsb[:], scr[m][:, :], mybir.AxisListType.X)
                nc.sync.dma_start(den_d[m], dsb[:])

            # ---- rescale (m=0 on DVE, m=1 on ACT) + transposes, per chunk ----
            t = 0
            for ci, (coff, csz) in enumerate(chunks):
                w0 = wc_t[0][ci][:, :csz]
                nc.vector.tensor_scalar_mul(w0, w0, fac[0][:, ci:ci + 1])
                w1 = wc_t[1][ci][:, :csz]
                nc.scalar.activation(
                    w1, w1, mybir.ActivationFunctionType.Copy,
                    bias=0.0, scale=fac[1][:, ci:ci + 1])
                for lo in range(0, ceil128(csz), 128):
                    for m in range(M_TILES):
                        tp = tps.tile([128, 128], dt.float16, tag="tp")
                        nc.tensor.matmul(tp[:], wc_t[m][ci][:, lo:lo + 128],
                                         ident[:, :], is_transpose=True,
                                         start=True, stop=True)
                        nc.vector.tensor_copy(wt_t[t][:, m * 128:(m + 1) * 128], tp[:])
                    t += 1
            assert t == ki_n

            # ---------------- GEMM2: num = W @ y ----------------
            for dci in range(DCH2):
                yn = yn_t[dci % YNBUF]
                for m in range(M_TILES):
                    ps = g1ps.tile([128, CH1], dt.float32, tag="g1ps")
                    psv = ps[:, :G2CH]
                    for ki in range(ki_n):
                        nc.tensor.matmul(
                            psv, wt_t[ki][:, m * 128:(m + 1) * 128],
                            yn[:, ki, :],
                            start=(ki == 0), stop=(ki == ki_n - 1))
                    o = outp.tile([128, G2CH], dt.float32, tag="odrain")
                    nc.vector.tensor_copy(o[:], psv)
                    nc.sync.dma_start(num_d[m][:, dci * G2CH:(dci + 1) * G2CH], o[:])
                if dci + YNBUF < DCH2:
                    yn_dma(dci % YNBUF, dci + YNBUF)

    nc.compile()
    return nc


def prep_inputs(input, sigma, train_data, n_cores=N_CORES):
    """Host-side shard + pre-tile. Returns list of per-core in_maps."""
    x = np.asarray(input, dtype=np.float32).reshape(B, D)
    sig = np.asarray(sigma, dtype=np.float64)
    y = np.asarray(train_data, dtype=np.float32).reshape(N_TOTAL, D)

    x16 = x.astype(np.float16)
    # xs[m, k, p(d), b]
    xs = np.empty((M_TILES, KT, 128, 128), dtype=np.float16)
    for m in range(M_TILES):
        for k in range(KT):
            xs[m, k] = x16[m * 128:(m + 1) * 128, k * 128:(k + 1) * 128].T
    is2 = (1.0 / sig ** 2).astype(np.float32).reshape(M_TILES, 128)

    ns = N_TOTAL // n_cores
    chunks = chunk_list(ns)
    ki_n = (ns + 127) // 128
    ns_pad = ki_n * 128

    in_maps = []
    for c in range(n_cores):
        ys = y[c * ns:(c + 1) * ns]
        ysh = ys.astype(np.float16)
        y2 = (-0.5 * np.einsum("ij,ij->i", ys.astype(np.float64), ys.astype(np.float64)))
        y2h = y2.astype(np.float16)
        y2l = (y2 - y2h.astype(np.float64)).astype(np.float16)
        y2hl = np.stack([y2h, y2l])            # [2, ns]
        ynat = np.zeros((ns_pad, D), dtype=np.float16)
        ynat[:ns] = ysh
        im = {"xs": xs, "invsig2": is2, "y2hl": y2hl, "ynat": ynat}
        for ci, (coff, csz) in enumerate(chunks):
            im[f"yh_c{ci}"] = np.ascontiguousarray(ysh[coff:coff + csz].T).reshape(KT, 128, csz)
        in_maps.append(im)
    return in_maps


def combine(results):
    """Flash-style combine of per-core (num, den, mx) partials -> full output."""
    num = np.stack([r["num"].reshape(B, D) for r in results]).astype(np.float64)
    den = np.stack([r["den"].reshape(B) for r in results]).astype(np.float64)
    mx = np.stack([r["mx"].reshape(B) for r in results]).astype(np.float64)
    M = mx.max(axis=0)
    r = np.exp(mx - M[None, :])
    num_tot = (num * r[:, :, None]).sum(axis=0)
    den_tot = (den * r).sum(axis=0)
    out = (num_tot / den_tot[:, None]).astype(np.float32)
    return out.reshape(B, C, H, W_IMG)


_NC_CACHE = {}


def get_nc(ns=NS):
    if ns not in _NC_CACHE:
        _NC_CACHE[ns] = build_nc(ns)
    return _NC_CACHE[ns]


def kernel(input, sigma, train_data):
    nc = get_nc()
    in_maps = prep_inputs(input, sigma, train_data)
    res = run_bass_kernel_spmd(nc, in_maps, core_ids=list(range(N_CORES)))
    return combine(res.results)


# revision 13
# speedup vs baseline: 1.0018x; 1.0018x over previous
"""Karras optimal denoiser on 8 Trainium2 NeuronCores.

Math: D(x, sigma) = softmax_i(L_bi) @ y  with  L_bi = (x_b . y_i - 0.5||y_i||^2) / sigma_b^2
(the per-row constant -0.5||x||^2/sigma^2 cancels in softmax).

Sharding: train_data split over N across 8 cores. Each core returns
(num, den, mx) = (sum_i w y_i, sum_i w, rowmax L * invsig2) in its local max
frame; host does the flash-style combine.

Per-core kernel:
  GEMM1 (logits): single-pass fp16  x.y  accumulated in fp32 PSUM (CPU sim of
    this exact pipeline gives rel err ~3e-3 vs the 2e-2 gate); -0.5||y||^2 is
    added via a K=2 fp16 ones-matmul broadcast of the (hi,lo) fp16 split of y2.
  Softmax: per-chunk DVE rowmax + ACT exp (scale=1/sigma^2, bias=-chunkmax/sigma^2,
    accum_out=rowsum); after all chunks, per-chunk correction factors
    exp((cm_c - gm)/sigma^2) rescale W (m=0 on DVE, m=1 on ACT in parallel),
    each chunk's transposes issued right behind its rescale.
  W^T via PE transpose-mode (fp16), then GEMM2 (num = W @ y) with W^T stationary
    and native-layout fp16 y as the moving operand, triple-buffered; the first
    two y-chunks stream in small pieces during GEMM1 (phase-1 DMA is ~80%
    saturated by the y^T stream, lumpy prefetch stalls the PE), the third at
    the tail, the rest 3-deep during GEMM2.
"""

import numpy as np
from contextlib import ExitStack

import concourse.bass as bass
import concourse.tile as tile
import concourse.mybir as mybir
from concourse import bacc
from concourse.bass_utils import run_bass_kernel_spmd
from concourse.masks import make_identity

dt = mybir.dt

B, C, H, W_IMG = 256, 3, 32, 32
D = C * H * W_IMG            # 3072
N_TOTAL = 50000
N_CORES = 8
NS = N_TOTAL // N_CORES      # 6250 per core
KT = D // 128                # 24 contraction k-tiles
HK = KT // 2                 # 12 k-tiles per xs half (startup pipelining)
M_TILES = 2                  # 256 query rows = 2 partition tiles
CH1 = 384                    # GEMM1 chunk width (SBUF-driven)
G2CH = 256                   # GEMM2 output-d chunk width
DCH2 = D // G2CH             # 12 output-d chunks for GEMM2
YNBUF = 4                    # GEMM2 y stream depth


def ceil128(v):
    return (v + 127) // 128 * 128


def chunk_list(ns):
    """Leading small chunks shrink the first-matmul DMA wait."""
    out = []
    off = 0
    for sz in (128, 128, 256):
        out.append((off, sz))
        off += sz
    while off < ns:
        sz = min(CH1, ns - off)
        out.append((off, sz))
        off += sz
    return out


def build_nc(ns=NS):
    """Build + compile the per-core Bass program (parameterized by shard size)."""
    chunks = chunk_list(ns)
    nch = len(chunks)
    ki_n = (ns + 127) // 128          # ki tiles for GEMM2 (ynat zero-padded)
    ns_pad = ki_n * 128
    assert sum(ceil128(csz) for _, csz in chunks) == ns_pad
    # ki tile -> (chunk idx, col offset inside chunk tile)
    ki_map = []
    for ci, (coff, csz) in enumerate(chunks):
        for lo in range(0, ceil128(csz), 128):
            ki_map.append((ci, lo))
    assert len(ki_map) == ki_n

    nc = bacc.Bacc("TRN2", target_bir_lowering=False, debug=False)

    # --- DRAM I/O ---
    yh_d = [nc.dram_tensor(f"yh_c{ci}", (KT, 128, csz), dt.float16, kind="ExternalInput").ap()
            for ci, (_, csz) in enumerate(chunks)]
    y2_d = nc.dram_tensor("y2hl", (2, ns), dt.float16, kind="ExternalInput").ap()
    xs_d = nc.dram_tensor("xs", (M_TILES, KT, 128, 128), dt.float16, kind="ExternalInput").ap()
    yn_d = nc.dram_tensor("ynat", (ns_pad, D), dt.float16, kind="ExternalInput").ap()
    is2_d = nc.dram_tensor("invsig2", (M_TILES, 128), dt.float32, kind="ExternalInput").ap()

    num_d = nc.dram_tensor("num", (M_TILES, 128, D), dt.float16, kind="ExternalOutput").ap()
    den_d = nc.dram_tensor("den", (M_TILES, 128, 1), dt.float32, kind="ExternalOutput").ap()
    mx_d = nc.dram_tensor("mx", (M_TILES, 128, 1), dt.float32, kind="ExternalOutput").ap()

    with tile.TileContext(nc) as tc:
        with ExitStack() as ctx:
            small = ctx.enter_context(tc.tile_pool(name="small", bufs=1))
            xpool = ctx.enter_context(tc.tile_pool(name="x", bufs=1))
            wpool = ctx.enter_context(tc.tile_pool(name="w", bufs=1))
            wtpool = ctx.enter_context(tc.tile_pool(name="wt", bufs=1))
            ynpool = ctx.enter_context(tc.tile_pool(name="yn", bufs=1))
            biasp = ctx.enter_context(tc.tile_pool(name="bias", bufs=8))
            g1ps = ctx.enter_context(tc.tile_pool(name="g1ps", bufs=4, space="PSUM"))
            outp = ctx.enter_context(tc.tile_pool(name="odrain", bufs=4))
            ypool = ctx.enter_context(tc.tile_pool(name="ystream", bufs=2))
            y2pool = ctx.enter_context(tc.tile_pool(name="y2p", bufs=2))

            # ---- chunk-0 stream DMAs first so the PE can start ASAP ----
            def load_chunk(ci):
                coff, csz = chunks[ci]
                yh_t = ypool.tile([128, KT, csz], dt.float16, tag="yh")
                y2_t = y2pool.tile([2, csz], dt.float16, tag="y2")
                nc.sync.dma_start(yh_t[:], yh_d[ci].rearrange("k p s -> p k s"))
                nc.sync.dma_start(y2_t[:], y2_d[:, coff:coff + csz])
                return yh_t, y2_t

            pending = load_chunk(0)

            # constants / small state
            is2_t = small.tile([128, M_TILES], dt.float32, tag="is2")
            nc.sync.dma_start(is2_t[:], is2_d.rearrange("m p -> p m"))
            xs_t = [[xpool.tile([128, HK, 128], dt.float16, tag=f"xs{m}h{h}",
                                name=f"xs{m}h{h}") for h in range(2)]
                    for m in range(M_TILES)]
            for m in range(M_TILES):
                for h in range(2):
                    nc.sync.dma_start(xs_t[m][h][:],
                                      xs_d[m, h * HK:(h + 1) * HK].rearrange("k p b -> p k b"))

            ones2 = small.tile([2, 128], dt.float16, tag="ones2")
            nc.vector.memset(ones2[:], 1.0)
            cm_st = [small.tile([128, nch], dt.float32, tag=f"cm{m}", name=f"cm{m}") for m in range(M_TILES)]
            s_st = [small.tile([128, nch], dt.float32, tag=f"ss{m}", name=f"ss{m}") for m in range(M_TILES)]
            fac = [small.tile([128, nch], dt.float32, tag=f"fac{m}", name=f"fac{m}") for m in range(M_TILES)]
            scr = [small.tile([128, nch], dt.float32, tag=f"scr{m}", name=f"scr{m}") for m in range(M_TILES)]
            # per-chunk W tiles (fine-grained rescale -> transpose pipelining)
            wc_t = [[wpool.tile([128, ceil128(csz)], dt.float16, tag=f"W{m}c{ci}",
                                name=f"W{m}c{ci}") for ci, (_, csz) in enumerate(chunks)]
                    for m in range(M_TILES)]
            wt_all = wtpool.tile([128, ki_n * M_TILES * 128], dt.float16, tag="wtall")
            yn_t = [ynpool.tile([128, ki_n, G2CH], dt.float16, tag=f"yn{j}", name=f"yn{j}")
                    for j in range(YNBUF)]

            for m in range(M_TILES):
                for ci, (_, csz) in enumerate(chunks):
                    if ceil128(csz) > csz:
                        nc.vector.memset(wc_t[m][ci][:, csz:], 0.0)

            def yn_dma(j, dci, ki0=0, ki1=ki_n):
                nc.sync.dma_start(
                    yn_t[j][:, ki0:ki1, :],
                    yn_d[ki0 * 128:ki1 * 128,
                         dci * G2CH:(dci + 1) * G2CH].rearrange(
                        "(ki p) s -> p ki s", p=128))

            # spread the first GEMM2 y-chunk load thinly across phase 1
            # (phase-1 DMA runs at ~90% of roofline; lumpy prefetch stalls PE)
            pieces = []                 # (buf j, dci, ki0, ki1)
            for j, dci in ((0, 0), (1, 1)):
                for s in range(5):
                    ki0 = (ki_n * s) // 5
                    ki1 = (ki_n * (s + 1)) // 5
                    if ki1 > ki0:
                        pieces.append((j, dci, ki0, ki1))
            first_pf = 6                # chunk idx of first prefetch piece

            # ---------------- GEMM1 + per-chunk softmax pieces ----------------
            for ci, (coff, csz) in enumerate(chunks):
                yh_t, y2_t = pending
                if ci + 1 < nch:
                    pending = load_chunk(ci + 1)
                if first_pf <= ci < first_pf + len(pieces):
                    yn_dma(*pieces[ci - first_pf])

                for m in range(M_TILES):
                    ps = g1ps.tile([128, CH1], dt.float32, tag="g1ps")
                    psv = ps[:, :csz]
                    # exact -0.5*||y||^2 broadcast: ones[2,128].T @ y2hl[2,csz] (fp16)
                    nc.tensor.matmul(psv, ones2[:, :], y2_t[:, :], start=True, stop=False)
                    for k in range(KT):
                        nc.tensor.matmul(
                            psv,
                            xs_t[m][k // HK][:, k % HK, :],
                            yh_t[:, k, :],
                            start=False,
                            stop=(k == KT - 1),
                        )
                    cmsl = cm_st[m][:, ci:ci + 1]
                    nc.vector.reduce_max(cmsl, psv, mybir.AxisListType.X)
                    bias_t = biasp.tile([128, 1], dt.float32, tag="bias")
                    nc.vector.tensor_scalar(
                        bias_t[:], cmsl, is2_t[:, m:m + 1], -1.0,
                        op0=mybir.AluOpType.mult, op1=mybir.AluOpType.mult)
                    nc.scalar.activation(
                        wc_t[m][ci][:, :csz], psv,
                        mybir.ActivationFunctionType.Exp,
                        bias=bias_t[:], scale=is2_t[:, m:m + 1],
                        accum_out=s_st[m][:, ci:ci + 1])

            # issue any prefetch pieces that didn't fit in the chunk loop
            for p in pieces[max(0, nch - first_pf):]:
                yn_dma(*p)
            # third GEMM2 y chunk ahead of the transposes (it is needed first)
            yn_dma(2, 2)

            # ------------- global max, factors, den/mx -------------
            for m in range(M_TILES):
                gm = small.tile([128, 1], dt.float32, tag=f"gm{m}")
                nc.vector.reduce_max(gm[:], cm_st[m][:, :], mybir.AxisListType.X)
                neg = small.tile([128, 1], dt.float32, tag=f"neg{m}")
                nc.vector.tensor_scalar(
                    neg[:], gm[:], is2_t[:, m:m + 1], -1.0,
                    op0=mybir.AluOpType.mult, op1=mybir.AluOpType.mult)
                mxs = small.tile([128, 1], dt.float32, tag=f"mxs{m}")
                nc.vector.tensor_scalar_mul(mxs[:], neg[:], -1.0)
                nc.sync.dma_start(mx_d[m], mxs[:])
                nc.scalar.activation(
                    fac[m][:, :], cm_st[m][:, :],
                    mybir.ActivationFunctionType.Exp,
                    bias=neg[:], scale=is2_t[:, m:m + 1])
                dsb = small.tile([128, 1], dt.float32, tag=f"den{m}")
                nc.vector.tensor_mul(scr[m][:, :], s_st[m][:, :], fac[m][:, :])
                nc.vector.reduce_sum(dsb[:], scr[m][:, :], mybir.AxisListType.X)
                nc.sync.dma_start(den_d[m], dsb[:])

            # ---- rescale (m=0 on DVE, m=1 on ACT) + DMA-engine transposes ----
            t = 0
            for ci, (coff, csz) in enumerate(chunks):
                w0 = wc_t[0][ci][:, :csz]
                nc.vector.tensor_scalar_mul(w0, w0, fac[0][:, ci:ci + 1])
                w1 = wc_t[1][ci][:, :csz]
                nc.scalar.activation(
                    w1, w1, mybir.ActivationFunctionType.Copy,
                    bias=0.0, scale=fac[1][:, ci:ci + 1])
                for lo in range(0, ceil128(csz), 128):
                    for m in range(M_TILES):
                        nc.sync.dma_start_transpose(
                            wt_all[:, (t * M_TILES + m) * 128:(t * M_TILES + m + 1) * 128],
                            wc_t[m][ci][:, lo:lo + 128])
                    t += 1
                if ci == nch // 2:
                    yn_dma(3, 3)   # fourth GEMM2 y chunk mid-tail
            assert t == ki_n

            # ---------------- GEMM2: num = W @ y ----------------
            for dci in range(DCH2):
                yn = yn_t[dci % YNBUF]
                for m in range(M_TILES):
                    ps = g1ps.tile([128, CH1], dt.float32, tag="g1ps")
                    psv = ps[:, :G2CH]
                    for ki in range(ki_n):
                        nc.tensor.matmul(
                            psv, wt_all[:, (ki * M_TILES + m) * 128:(ki * M_TILES + m + 1) * 128],
                            yn[:, ki, :],
                            start=(ki == 0), stop=(ki == ki_n - 1))
                    o = outp.tile([128, G2CH], dt.float16, tag="odrain")
                    nc.vector.tensor_copy(o[:], psv)
                    nc.sync.dma_start(num_d[m][:, dci * G2CH:(dci + 1) * G2CH], o[:])
                if dci + YNBUF < DCH2:
                    yn_dma(dci % YNBUF, dci + YNBUF)

    nc.compile()
    return nc


def prep_inputs(input, sigma, train_data, n_cores=N_CORES):
    """Host-side shard + pre-tile. Returns list of per-core in_maps."""
    x = np.asarray(input, dtype=np.float32).reshape(B, D)
    sig = np.asarray(sigma, dtype=np.float64)
    y = np.asarray(train_data, dtype=np.float32).reshape(N_TOTAL, D)

    x16 = x.astype(np.float16)
    # xs[m, k, p(d), b]
    xs = np.empty((M_TILES, KT, 128, 128), dtype=np.float16)
    for m in range(M_TILES):
        for k in range(KT):
            xs[m, k] = x16[m * 128:(m + 1) * 128, k * 128:(k + 1) * 128].T
    is2 = (1.0 / sig ** 2).astype(np.float32).reshape(M_TILES, 128)

    ns = N_TOTAL // n_cores
    chunks = chunk_list(ns)
    ki_n = (ns + 127) // 128
    ns_pad = ki_n * 128

    in_maps = []
    for c in range(n_cores):
        ys = y[c * ns:(c + 1) * ns]
        ysh = ys.astype(np.float16)
        y2 = (-0.5 * np.einsum("ij,ij->i", ys.astype(np.float64), ys.astype(np.float64)))
        y2h = y2.astype(np.float16)
        y2l = (y2 - y2h.astype(np.float64)).astype(np.float16)
        y2hl = np.stack([y2h, y2l])            # [2, ns]
        ynat = np.zeros((ns_pad, D), dtype=np.float16)
        ynat[:ns] = ysh
        im = {"xs": xs, "invsig2": is2, "y2hl": y2hl, "ynat": ynat}
        for ci, (coff, csz) in enumerate(chunks):
            im[f"yh_c{ci}"] = np.ascontiguousarray(ysh[coff:coff + csz].T).reshape(KT, 128, csz)
        in_maps.append(im)
    return in_maps


def combine(results):
    """Flash-style combine of per-core (num, den, mx) partials -> full output."""
    num = np.stack([r["num"].reshape(B, D) for r in results]).astype(np.float64)
    den = np.stack([r["den"].reshape(B) for r in results]).astype(np.float64)
    mx = np.stack([r["mx"].reshape(B) for r in results]).astype(np.float64)
    M = mx.max(axis=0)
    r = np.exp(mx - M[None, :])
    num_tot = (num * r[:, :, None]).sum(axis=0)
    den_tot = (den * r).sum(axis=0)
    out = (num_tot / den_tot[:, None]).astype(np.float32)
    return out.reshape(B, C, H, W_IMG)


_NC_CACHE = {}


def get_nc(ns=NS):
    if ns not in _NC_CACHE:
        _NC_CACHE[ns] = build_nc(ns)
    return _NC_CACHE[ns]


def kernel(input, sigma, train_data):
    nc = get_nc()
    in_maps = prep_inputs(input, sigma, train_data)
    res = run_bass_kernel_spmd(nc, in_maps, core_ids=list(range(N_CORES)))
    return combine(res.results)
